# revision 1
# baseline (speedup 1.0000x reference)
"""Trainium2 Bass kernel for a leaky-integrate-fire (LIF) scan.

Reference computation (forward values only):
    v_t   = mem_{t-1} * 0.25 + x_t          (mem_0 carry = 0)
    s_t   = (v_t > 1.0) ? 1.0 : 0.0         (spike, the output)
    mem_t = (v_t <= 1.0) * v_t              (hard reset)

x: [T=32, B=64, N=16384] f32. Elementwise over (B, N), sequential over T.
Sharding: data-parallel over flattened B*N across 8 cores. Each core's slab
is laid out [P=128, T=32, F=1024] in DRAM so every partition's time series
is contiguous (16 KB DMA descriptor lines instead of 4 KB).

Design notes:
- The reset needs no spike tensor: mem = (v <= 1) * v is one
  scalar_tensor_tensor with in0 == in1 == v, so the serial time chain is just
  2 DVE ops per step. (scalar_tensor_tensor is DVE-only; the Pool engine
  fails the ISA engine check for it.)
- gpsimd chains the last W=208 columns using Pool-legal ops only
  (tensor_scalar / tensor_tensor, 4 ops per step) to offload the DVE.
- Each block's load is split across both HWDGE rings (sync + scalar) so the
  two descriptor queues drain load traffic in parallel; Sign batches 2 steps
  per instruction to amortize the ACT fixed cost.
- The spike compare is a pure function of v and runs off the chain on the
  scalar engine as Sign(v - 1) -> int8 in {-1, 0, 1}. For v in [0.5, 2),
  v - 1 is exact (Sterbenz), so sign(v-1) == 1  <=>  v > 1 exactly. The host
  maps (raw == 1) -> 1.0f, which also handles the v == 1 boundary the same
  way the reference does. int8 stores cut store traffic 4x.
- All on-device arithmetic is exactly reproducible fp32 (decay is a power of
  two, resets multiply by exactly 0.0/1.0), so the kernel matches the jax
  reference bitwise.
"""

import numpy as np

T = 32
B = 64
N = 16384
NCORES = 8
P = 128                      # SBUF partitions
F = (B // NCORES) * N // P   # 1024 free-dim columns per step per core
TB = 4                       # timesteps per DMA block (2 MiB loads)
SB = 2                       # timesteps per Sign batch (ACT fixed-cost amortize)
W = 208                      # columns whose chain runs on gpsimd
DECAY = 0.25
VTH = 1.0

_CACHE = {}


def _build_program():
    import concourse.bacc as bacc
    import concourse.tile as tile
    from concourse import mybir

    nc = bacc.Bacc(
        target_bir_lowering=False,
        debug=False,
        enable_asserts=False,
        num_devices=NCORES,
    )
    f32 = mybir.dt.float32
    i8 = mybir.dt.int8
    Alu = mybir.AluOpType
    Act = mybir.ActivationFunctionType
    D = F - W

    x_d = nc.dram_tensor("x", [P, T, F], f32, kind="ExternalInput").ap()
    o_d = nc.dram_tensor("out", [P, T, F], i8, kind="ExternalOutput").ap()

    with tile.TileContext(nc) as tc:
        with (
            tc.tile_pool(name="xp", bufs=3) as xpool,
            tc.tile_pool(name="sp", bufs=3) as spool,
            tc.tile_pool(name="vp", bufs=3) as vpool,
            tc.tile_pool(name="mp", bufs=1) as mpool,
        ):
            mem = mpool.tile([P, F], f32)    # DVE-owned membrane carry
            nc.vector.memset(mem[:], 0.0)
            nbias = mpool.tile([P, 1], f32)  # per-partition bias = -VTH
            nc.vector.memset(nbias[:], -VTH)
            memg = mpool.tile([P, W], f32)  # gpsimd-owned membrane carry
            nc.gpsimd.memset(memg[:], 0.0)
            for blk in range(T // TB):
                xt = xpool.tile([P, TB, F], f32)
                # split each block's load across both HWDGE rings
                h = TB // 2
                nc.sync.dma_start(
                    out=xt[:, :h], in_=x_d[:, blk * TB:blk * TB + h, :])
                nc.scalar.dma_start(
                    out=xt[:, h:], in_=x_d[:, blk * TB + h:(blk + 1) * TB, :])
                st = spool.tile([P, TB, F], i8)
                for g in range(TB // SB):
                    # v tiles rotate so the off-chain compare can overlap the
                    # chain of later steps without a WAR stall; SB steps share
                    # one v tile so a single Sign covers them.
                    v = vpool.tile([P, SB, F], f32)
                    for jj in range(SB):
                        j = g * SB + jj
                        # DVE chain, columns [0, D)
                        nc.vector.scalar_tensor_tensor(
                            out=v[:, jj, :D], in0=mem[:, :D], scalar=DECAY,
                            in1=xt[:, j, :D], op0=Alu.mult, op1=Alu.add,
                        )
                        nc.vector.scalar_tensor_tensor(
                            out=mem[:, :D], in0=v[:, jj, :D], scalar=VTH,
                            in1=v[:, jj, :D], op0=Alu.is_le, op1=Alu.mult,
                        )
                        # gpsimd chain, columns [D, F), Pool-legal ops only
                        dg = vpool.tile([P, W], f32, name="dg")
                        nc.gpsimd.tensor_scalar(
                            out=dg[:], in0=memg[:], scalar1=DECAY,
                            scalar2=None, op0=Alu.mult,
                        )
                        nc.gpsimd.tensor_tensor(
                            out=v[:, jj, D:], in0=dg[:], in1=xt[:, j, D:],
                            op=Alu.add,
                        )
                        kg = vpool.tile([P, W], f32, name="kg")
                        nc.gpsimd.tensor_scalar(
                            out=kg[:], in0=v[:, jj, D:], scalar1=VTH,
                            scalar2=None, op0=Alu.is_le,
                        )
                        nc.gpsimd.tensor_tensor(
                            out=memg[:], in0=kg[:], in1=v[:, jj, D:],
                            op=Alu.mult,
                        )
                    # raw spikes = Sign(v - VTH) in {-1,0,1} for SB steps
                    nc.scalar.activation(st[:, g * SB:(g + 1) * SB, :], v[:],
                                         Act.Sign, bias=nbias[:])
                nc.scalar.dma_start(out=o_d[:, blk * TB:(blk + 1) * TB, :], in_=st[:])
    nc.compile()
    return nc


def _get_nc():
    if "nc" not in _CACHE:
        _CACHE["nc"] = _build_program()
    return _CACHE["nc"]


def _get_runner():
    """Cache one jitted SPMD executable (same lowering as
    bass_utils.run_bass_kernel_spmd's axon path, which builds a fresh
    jax.jit closure per call and would recompile every time)."""
    if "runner" in _CACHE:
        return _CACHE["runner"]

    import jax
    from jax.sharding import Mesh, PartitionSpec
    from jax.experimental.shard_map import shard_map
    from concourse import bass2jax

    nc = _get_nc()
    bass2jax.install_neuronx_cc_hook()

    # operand order: real inputs, donated output buffers, partition_id last
    in_names = ("x", "out", "partition_id")
    out_names = ("out",)
    out_avals = (jax.core.ShapedArray((P, T, F), np.int8),)

    def _body(*args):
        outs = bass2jax._bass_exec_p.bind(
            *args,
            bass2jax.partition_id_tensor(),
            out_avals=out_avals,
            in_names=in_names,
            out_names=out_names,
            lowering_input_output_aliases=(),
            sim_require_finite=True,
            sim_require_nnan=True,
            nc=nc,
        )
        return tuple(outs)

    devices = jax.devices()[:NCORES]
    mesh = Mesh(np.asarray(devices), ("core",))
    sharded = jax.jit(
        shard_map(
            _body,
            mesh=mesh,
            in_specs=(PartitionSpec("core"),) * 2,
            out_specs=(PartitionSpec("core"),),
            check_rep=False,
        ),
        donate_argnums=(1,),
        keep_unused=True,
    )
    _CACHE["runner"] = sharded
    return sharded


def _run_sharded(x_concat):
    """x_concat: [NCORES*P, T, F] host array, core k's slab at rows k*P:(k+1)*P."""
    runner = _get_runner()
    zeros = np.zeros((NCORES * P, T, F), np.int8)
    (out,) = runner(x_concat, zeros)
    return np.asarray(out)


def kernel(x):
    x = np.asarray(x, dtype=np.float32)
    assert x.shape == (T, B, N), x.shape
    # [T, B, N] -> [T, 8, P, F] -> per-core [8, P, T, F] -> concat on axis 0
    x_concat = np.ascontiguousarray(
        x.reshape(T, NCORES, P, F).transpose(1, 2, 0, 3)
    ).reshape(NCORES * P, T, F)
    out = _run_sharded(x_concat)
    # [8*P, T, F] -> [8, P, T, F] -> [T, 8, P, F] -> [T, B, N]
    out = np.ascontiguousarray(
        out.reshape(NCORES, P, T, F).transpose(2, 0, 1, 3)
    ).reshape(T, B, N)
    # raw == 1 <=> v > VTH; exact 0.0/1.0 reconstruction
    return (out == 1).astype(np.float32)



# revision 2
# speedup vs baseline: 1.0649x; 1.0649x over previous
"""Trainium2 Bass kernel for a leaky-integrate-fire (LIF) scan.

Reference computation (forward values only):
    v_t   = mem_{t-1} * 0.25 + x_t          (mem_0 carry = 0)
    s_t   = (v_t > 1.0) ? 1.0 : 0.0         (spike, the output)
    mem_t = (v_t <= 1.0) * v_t              (hard reset)

x: [T=32, B=64, N=16384] f32. Elementwise over (B, N), sequential over T.
Sharding: data-parallel over flattened B*N across 8 cores. Each core's slab
is laid out [P=128, T=32, F=1024] in DRAM so every partition's time series
is contiguous (16 KB DMA descriptor lines instead of 4 KB).

Design notes (v2):
- Engine balance from the TimelineSim trace: the DMA_ENGINES device is the
  hard floor (16.78 MB load + 4.19 MB int8 store at 360 B/ns = 58.3 us), so
  both chain engines are sized to sit just under it.
- DVE chain (cols [0, D)): 2 scalar_tensor_tensor ops per step
  (v = mem*0.25 + x; mem = (v<=1)*v), 2.08 ns/col/step.
- Pool chain (cols [D, F)): carries mem4 = 0.25*mem so each step is 3 ops
  (v = mem4 + x; k4 = (v<=1)*0.25 via two-scalar tensor_scalar;
  mem4 = k4*v), 5.36 ns/col/step. The split 768/256 puts both engines at
  ~53-55 us, under the DMA floor.
- The spike compare is off-chain on ACT: Sign(v - 1) -> int8 in {-1,0,1},
  batched 2 steps per instruction. For v in [0.5, 2), v - 1 is exact
  (Sterbenz), so sign(v-1) == 1 <=> v > 1 exactly; the host maps
  (raw == 1) -> 1.0f. int8 stores cut store traffic 4x.
- Block-0 loads are issued per step (0.5 MB each, alternating HWDGE rings)
  so the chain starts after ~1.5 us instead of ~5.8; later blocks load
  2 MB split across the sync+scalar rings. The last block's store is split
  so the drain tail is one 0.25 MB transfer.
- All on-device arithmetic is exactly reproducible fp32 (decay is a power of
  two, resets multiply by exactly 0.0/0.25), so the kernel matches the jax
  reference bitwise.
"""

import numpy as np

T = 32
B = 64
N = 16384
NCORES = 8
P = 128                      # SBUF partitions
F = (B // NCORES) * N // P   # 1024 free-dim columns per step per core
TB = 4                       # timesteps per DMA block (2 MiB loads)
SB = 2                       # timesteps per Sign batch (ACT fixed-cost amortize)
W = 256                      # columns whose chain runs on the Pool engine
DECAY = 0.25
VTH = 1.0

_CACHE = {}


def _build_program():
    import concourse.bacc as bacc
    import concourse.tile as tile
    from concourse import mybir

    nc = bacc.Bacc(
        target_bir_lowering=False,
        debug=False,
        enable_asserts=False,
        num_devices=NCORES,
    )
    f32 = mybir.dt.float32
    i8 = mybir.dt.int8
    Alu = mybir.AluOpType
    Act = mybir.ActivationFunctionType
    D = F - W

    x_d = nc.dram_tensor("x", [P, T, F], f32, kind="ExternalInput").ap()
    o_d = nc.dram_tensor("out", [P, T, F], i8, kind="ExternalOutput").ap()

    with tile.TileContext(nc) as tc:
        with (
            tc.tile_pool(name="xp", bufs=3) as xpool,
            tc.tile_pool(name="sp", bufs=3) as spool,
            tc.tile_pool(name="vp", bufs=3) as vpool,
            tc.tile_pool(name="mp", bufs=1) as mpool,
        ):
            mem = mpool.tile([P, D], f32)     # DVE-owned membrane carry
            nc.vector.memset(mem[:], 0.0)
            nbias = mpool.tile([P, 1], f32)   # per-partition bias = -VTH
            nc.vector.memset(nbias[:], -VTH)
            mem4 = mpool.tile([P, W], f32)    # Pool-owned carry, pre-scaled 0.25x
            nc.gpsimd.memset(mem4[:], 0.0)
            for blk in range(T // TB):
                xt = xpool.tile([P, TB, F], f32)
                if blk == 0:
                    # per-step loads so step 0 can start after one 0.5 MB DMA
                    for j in range(TB):
                        eng = nc.sync if j % 2 == 0 else nc.scalar
                        eng.dma_start(out=xt[:, j:j + 1], in_=x_d[:, j:j + 1, :])
                else:
                    h = TB // 2
                    nc.sync.dma_start(
                        out=xt[:, :h], in_=x_d[:, blk * TB:blk * TB + h, :])
                    nc.scalar.dma_start(
                        out=xt[:, h:], in_=x_d[:, blk * TB + h:(blk + 1) * TB, :])
                st = spool.tile([P, TB, F], i8)
                for g in range(TB // SB):
                    # v tiles rotate so the off-chain compare can overlap the
                    # chain of later steps without a WAR stall; SB steps share
                    # one v tile so a single Sign covers them.
                    v = vpool.tile([P, SB, F], f32)
                    for jj in range(SB):
                        j = g * SB + jj
                        # DVE chain, columns [0, D): 2 fused ops per step
                        nc.vector.scalar_tensor_tensor(
                            out=v[:, jj, :D], in0=mem[:], scalar=DECAY,
                            in1=xt[:, j, :D], op0=Alu.mult, op1=Alu.add,
                        )
                        nc.vector.scalar_tensor_tensor(
                            out=mem[:], in0=v[:, jj, :D], scalar=VTH,
                            in1=v[:, jj, :D], op0=Alu.is_le, op1=Alu.mult,
                        )
                        # Pool chain, columns [D, F): 3 ops per step on the
                        # pre-scaled carry (mem4 == 0.25*mem exactly)
                        nc.gpsimd.tensor_tensor(
                            out=v[:, jj, D:], in0=mem4[:], in1=xt[:, j, D:],
                            op=Alu.add,
                        )
                        k4 = vpool.tile([P, W], f32, name="k4")
                        nc.gpsimd.tensor_scalar(
                            out=k4[:], in0=v[:, jj, D:], scalar1=VTH,
                            scalar2=DECAY, op0=Alu.is_le, op1=Alu.mult,
                        )
                        nc.gpsimd.tensor_tensor(
                            out=mem4[:], in0=k4[:], in1=v[:, jj, D:],
                            op=Alu.mult,
                        )
                    # raw spikes = Sign(v - VTH) in {-1,0,1} for SB steps
                    nc.scalar.activation(st[:, g * SB:(g + 1) * SB, :], v[:],
                                         Act.Sign, bias=nbias[:])
                if blk == T // TB - 1:
                    # split the final store so the drain tail is short
                    nc.scalar.dma_start(
                        out=o_d[:, blk * TB:blk * TB + SB, :], in_=st[:, :SB])
                    nc.scalar.dma_start(
                        out=o_d[:, blk * TB + SB:(blk + 1) * TB, :], in_=st[:, SB:])
                else:
                    nc.scalar.dma_start(
                        out=o_d[:, blk * TB:(blk + 1) * TB, :], in_=st[:])
    nc.compile()
    return nc


def _get_nc():
    if "nc" not in _CACHE:
        _CACHE["nc"] = _build_program()
    return _CACHE["nc"]


def _get_runner():
    """Cache one jitted SPMD executable (same lowering as
    bass_utils.run_bass_kernel_spmd's axon path, which builds a fresh
    jax.jit closure per call and would recompile every time)."""
    if "runner" in _CACHE:
        return _CACHE["runner"]

    import jax
    from jax.sharding import Mesh, PartitionSpec
    from jax.experimental.shard_map import shard_map
    from concourse import bass2jax

    nc = _get_nc()
    bass2jax.install_neuronx_cc_hook()

    # operand order: real inputs, donated output buffers, partition_id last
    in_names = ("x", "out", "partition_id")
    out_names = ("out",)
    out_avals = (jax.core.ShapedArray((P, T, F), np.int8),)

    def _body(*args):
        outs = bass2jax._bass_exec_p.bind(
            *args,
            bass2jax.partition_id_tensor(),
            out_avals=out_avals,
            in_names=in_names,
            out_names=out_names,
            lowering_input_output_aliases=(),
            sim_require_finite=True,
            sim_require_nnan=True,
            nc=nc,
        )
        return tuple(outs)

    devices = jax.devices()[:NCORES]
    mesh = Mesh(np.asarray(devices), ("core",))
    sharded = jax.jit(
        shard_map(
            _body,
            mesh=mesh,
            in_specs=(PartitionSpec("core"),) * 2,
            out_specs=(PartitionSpec("core"),),
            check_rep=False,
        ),
        donate_argnums=(1,),
        keep_unused=True,
    )
    _CACHE["runner"] = sharded
    return sharded


def _run_sharded(x_concat):
    """x_concat: [NCORES*P, T, F] host array, core k's slab at rows k*P:(k+1)*P."""
    runner = _get_runner()
    zeros = np.zeros((NCORES * P, T, F), np.int8)
    (out,) = runner(x_concat, zeros)
    return np.asarray(out)


def kernel(x):
    x = np.asarray(x, dtype=np.float32)
    assert x.shape == (T, B, N), x.shape
    # [T, B, N] -> [T, 8, P, F] -> per-core [8, P, T, F] -> concat on axis 0
    x_concat = np.ascontiguousarray(
        x.reshape(T, NCORES, P, F).transpose(1, 2, 0, 3)
    ).reshape(NCORES * P, T, F)
    out = _run_sharded(x_concat)
    # [8*P, T, F] -> [8, P, T, F] -> [T, 8, P, F] -> [T, B, N]
    out = np.ascontiguousarray(
        out.reshape(NCORES, P, T, F).transpose(2, 0, 1, 3)
    ).reshape(T, B, N)
    # raw == 1 <=> v > VTH; exact 0.0/1.0 reconstruction
    return (out == 1).astype(np.float32)


# revision 4
# speedup vs baseline: 1.0678x; 1.0028x over previous
"""Trainium2 Bass kernel for a leaky-integrate-fire (LIF) scan.

Reference computation (forward values only):
    v_t   = mem_{t-1} * 0.25 + x_t          (mem_0 carry = 0)
    s_t   = (v_t > 1.0) ? 1.0 : 0.0         (spike, the output)
    mem_t = (v_t <= 1.0) * v_t              (hard reset)

x: [T=32, B=64, N=16384] f32. Elementwise over (B, N), sequential over T.
Sharding: data-parallel over flattened B*N across 8 cores. Each core's slab
is laid out [P=128, T=32, F=1024] in DRAM so every partition's time series
is contiguous (16 KB DMA descriptor lines instead of 4 KB).

Design notes (v2):
- Engine balance from the TimelineSim trace: the DMA_ENGINES device is the
  hard floor (16.78 MB load + 4.19 MB int8 store at 360 B/ns = 58.3 us), so
  both chain engines are sized to sit just under it.
- DVE chain (cols [0, D)): 2 scalar_tensor_tensor ops per step
  (v = mem*0.25 + x; mem = (v<=1)*v), 2.08 ns/col/step.
- Pool chain (cols [D, F)): carries mem4 = 0.25*mem so each step is 3 ops
  (v = mem4 + x; k4 = (v<=1)*0.25 via two-scalar tensor_scalar;
  mem4 = k4*v), 5.36 ns/col/step. The split 768/256 puts both engines at
  ~53-55 us, under the DMA floor.
- The spike compare is off-chain on ACT: Sign(v - 1) -> int8 in {-1,0,1},
  batched 2 steps per instruction. For v in [0.5, 2), v - 1 is exact
  (Sterbenz), so sign(v-1) == 1 <=> v > 1 exactly; the host maps
  (raw == 1) -> 1.0f. int8 stores cut store traffic 4x.
- Block-0 loads are issued per step (0.5 MB each, alternating HWDGE rings)
  so the chain starts after ~1.5 us instead of ~5.8; later blocks load
  2 MB split across the sync+scalar rings. The last block's store is split
  so the drain tail is one 0.25 MB transfer.
- All on-device arithmetic is exactly reproducible fp32 (decay is a power of
  two, resets multiply by exactly 0.0/0.25), so the kernel matches the jax
  reference bitwise.
"""

import numpy as np

T = 32
B = 64
N = 16384
NCORES = 8
P = 128                      # SBUF partitions
F = (B // NCORES) * N // P   # 1024 free-dim columns per step per core
TB = 4                       # timesteps per DMA block (2 MiB loads)
SB = 2                       # timesteps per Sign batch (ACT fixed-cost amortize)
W = 256                      # columns whose chain runs on the Pool engine
DECAY = 0.25
VTH = 1.0

_CACHE = {}


def _build_program():
    import concourse.bacc as bacc
    import concourse.tile as tile
    from concourse import mybir

    nc = bacc.Bacc(
        target_bir_lowering=False,
        debug=False,
        enable_asserts=False,
        num_devices=NCORES,
    )
    f32 = mybir.dt.float32
    i8 = mybir.dt.int8
    Alu = mybir.AluOpType
    Act = mybir.ActivationFunctionType
    D = F - W

    x_d = nc.dram_tensor("x", [P, T, F], f32, kind="ExternalInput").ap()
    o_d = nc.dram_tensor("out", [P, T, F], i8, kind="ExternalOutput").ap()

    with tile.TileContext(nc) as tc:
        with (
            tc.tile_pool(name="xp", bufs=3) as xpool,
            tc.tile_pool(name="sp", bufs=3) as spool,
            tc.tile_pool(name="vp", bufs=3) as vpool,
            tc.tile_pool(name="kp", bufs=3) as kpool,
            tc.tile_pool(name="mp", bufs=1) as mpool,
        ):
            mem = mpool.tile([P, D], f32)     # DVE-owned membrane carry
            nc.vector.memset(mem[:], 0.0)
            nbias = mpool.tile([P, 1], f32)   # per-partition bias = -VTH
            nc.vector.memset(nbias[:], -VTH)
            mem4 = mpool.tile([P, W], f32)    # Pool-owned carry, pre-scaled 0.25x
            nc.gpsimd.memset(mem4[:], 0.0)
            for blk in range(T // TB):
                xt = xpool.tile([P, TB, F], f32)
                if blk == 0:
                    # per-step loads so step 0 can start after one 0.5 MB DMA
                    for j in range(TB):
                        eng = nc.sync if j % 2 == 0 else nc.scalar
                        eng.dma_start(out=xt[:, j:j + 1], in_=x_d[:, j:j + 1, :])
                else:
                    h = TB // 2
                    nc.sync.dma_start(
                        out=xt[:, :h], in_=x_d[:, blk * TB:blk * TB + h, :])
                    nc.scalar.dma_start(
                        out=xt[:, h:], in_=x_d[:, blk * TB + h:(blk + 1) * TB, :])
                st = spool.tile([P, TB, F], i8)
                for g in range(TB // SB):
                    # separate per-engine v tiles so the DVE and Pool chains
                    # never share a tile (tile-granularity dep tracking would
                    # insert a cross-engine semaphore every step); tiles
                    # rotate so the off-chain compare can overlap the chain
                    # of later steps without a WAR stall.
                    vd = vpool.tile([P, SB, D], f32, name="vd")
                    vg = vpool.tile([P, SB, W], f32, name="vg")
                    for jj in range(SB):
                        j = g * SB + jj
                        last = j == T - 1 and blk == T // TB - 1
                        # DVE chain, columns [0, D): 2 fused ops per step
                        nc.vector.scalar_tensor_tensor(
                            out=vd[:, jj, :], in0=mem[:], scalar=DECAY,
                            in1=xt[:, j, :D], op0=Alu.mult, op1=Alu.add,
                        )
                        if not last:  # final membrane is never read
                            nc.vector.scalar_tensor_tensor(
                                out=mem[:], in0=vd[:, jj, :], scalar=VTH,
                                in1=vd[:, jj, :], op0=Alu.is_le, op1=Alu.mult,
                            )
                        # Pool chain, columns [D, F): 3 ops per step on the
                        # pre-scaled carry (mem4 == 0.25*mem exactly)
                        nc.gpsimd.tensor_tensor(
                            out=vg[:, jj, :], in0=mem4[:], in1=xt[:, j, D:],
                            op=Alu.add,
                        )
                        if not last:
                            k4 = kpool.tile([P, W], f32)
                            nc.gpsimd.tensor_scalar(
                                out=k4[:], in0=vg[:, jj, :], scalar1=VTH,
                                scalar2=DECAY, op0=Alu.is_le, op1=Alu.mult,
                            )
                            nc.gpsimd.tensor_tensor(
                                out=mem4[:], in0=k4[:], in1=vg[:, jj, :],
                                op=Alu.mult,
                            )
                    # raw spikes = Sign(v - VTH) in {-1,0,1} for SB steps,
                    # one call per engine slice
                    nc.scalar.activation(st[:, g * SB:(g + 1) * SB, :D], vd[:],
                                         Act.Sign, bias=nbias[:])
                    nc.scalar.activation(st[:, g * SB:(g + 1) * SB, D:], vg[:],
                                         Act.Sign, bias=nbias[:])
                if blk == T // TB - 1:
                    # split the final store so the drain tail is short
                    nc.scalar.dma_start(
                        out=o_d[:, blk * TB:blk * TB + SB, :], in_=st[:, :SB])
                    nc.scalar.dma_start(
                        out=o_d[:, blk * TB + SB:(blk + 1) * TB, :], in_=st[:, SB:])
                else:
                    nc.scalar.dma_start(
                        out=o_d[:, blk * TB:(blk + 1) * TB, :], in_=st[:])
    nc.compile()
    return nc


def _get_nc():
    if "nc" not in _CACHE:
        _CACHE["nc"] = _build_program()
    return _CACHE["nc"]


def _get_runner():
    """Cache one jitted SPMD executable (same lowering as
    bass_utils.run_bass_kernel_spmd's axon path, which builds a fresh
    jax.jit closure per call and would recompile every time)."""
    if "runner" in _CACHE:
        return _CACHE["runner"]

    import jax
    from jax.sharding import Mesh, PartitionSpec
    from jax.experimental.shard_map import shard_map
    from concourse import bass2jax

    nc = _get_nc()
    bass2jax.install_neuronx_cc_hook()

    # operand order: real inputs, donated output buffers, partition_id last
    in_names = ("x", "out", "partition_id")
    out_names = ("out",)
    out_avals = (jax.core.ShapedArray((P, T, F), np.int8),)

    def _body(*args):
        outs = bass2jax._bass_exec_p.bind(
            *args,
            bass2jax.partition_id_tensor(),
            out_avals=out_avals,
            in_names=in_names,
            out_names=out_names,
            lowering_input_output_aliases=(),
            sim_require_finite=True,
            sim_require_nnan=True,
            nc=nc,
        )
        return tuple(outs)

    devices = jax.devices()[:NCORES]
    mesh = Mesh(np.asarray(devices), ("core",))
    sharded = jax.jit(
        shard_map(
            _body,
            mesh=mesh,
            in_specs=(PartitionSpec("core"),) * 2,
            out_specs=(PartitionSpec("core"),),
            check_rep=False,
        ),
        donate_argnums=(1,),
        keep_unused=True,
    )
    _CACHE["runner"] = sharded
    return sharded


def _run_sharded(x_concat):
    """x_concat: [NCORES*P, T, F] host array, core k's slab at rows k*P:(k+1)*P."""
    runner = _get_runner()
    zeros = np.zeros((NCORES * P, T, F), np.int8)
    (out,) = runner(x_concat, zeros)
    return np.asarray(out)


def kernel(x):
    x = np.asarray(x, dtype=np.float32)
    assert x.shape == (T, B, N), x.shape
    # [T, B, N] -> [T, 8, P, F] -> per-core [8, P, T, F] -> concat on axis 0
    x_concat = np.ascontiguousarray(
        x.reshape(T, NCORES, P, F).transpose(1, 2, 0, 3)
    ).reshape(NCORES * P, T, F)
    out = _run_sharded(x_concat)
    # [8*P, T, F] -> [8, P, T, F] -> [T, 8, P, F] -> [T, B, N]
    out = np.ascontiguousarray(
        out.reshape(NCORES, P, T, F).transpose(2, 0, 1, 3)
    ).reshape(T, B, N)
    # raw == 1 <=> v > VTH; exact 0.0/1.0 reconstruction
    return (out == 1).astype(np.float32)
